# revision 8
# baseline (speedup 1.0000x reference)
"""MoE-LoRA kernel for Trainium2 (8 NeuronCores, Bass/Tile). v3

Math: per sample b (except the last, which is zeroed), with e = label[b]:
    out[b] = ALPHA * ( (x[b] @ A_e.T) @ B_e.T  +  (x[b] @ A_gen.T) @ B_gen.T )
The expert and general LoRA paths merge into a single rank-128 LoRA:
    Acat[b] = [A_e ; A_gen]          [2R, D]
    Bcat[b] = [B_e , B_gen]          [D, 2R]
    out[b]  = (x[b] @ Acat[b].T) @ (ALPHA * Bcat[b]).T

Key idea (v2): the host ships x PRE-TRANSPOSED (d-major bf16), so the device
runs only the two GEMM passes -- no PE transposes.  The kernel is DMA-bound:
~86 MB/core at ~400 GB/s.

v3: 512-row DMA granularity and edge ring-splitting -- during pipeline fill
only the input stream exists and during drain only the output stream, so
those phases use BOTH HWDGE rings (sync + scalar) to keep the SDMA pool fed.

Device pipeline per 512-row sub-block (software-pipelined by one stage):
    DMA xT sub-block in ([128 d-part, 10 k, 512 s])
    PE  GEMM1: hT[r=128, 512] += acatT_k.T @ xT_k      (10 D-chunks, PSUM)
    ACT evacuate hT -> SBUF bf16
    PE  GEMM2: out[s,d] = hT_st.T @ bctT   (N=512/512/256 into 3-bank PSUM)
    DVE/ACT evacuate out tiles -> SBUF bf16 (alternating)
    DMA out (512-row granularity)

Sharding: data-parallel over batch, 4 samples/core; per-sample LoRA tables
gathered + merged + transposed host-side, shipped bf16.
"""

import numpy as np
import ml_dtypes

import concourse.mybir as mybir
import concourse.tile as tile
from concourse import bacc
from concourse.bass import ts
from concourse.bass_utils import run_bass_kernel_spmd

# Problem shape (hardcoded; kernel.py must be self-contained).
B, S, D, R, E = 32, 4096, 1280, 64, 8
ALPHA = 2.0
NCORES = 8
NS = B // NCORES          # samples per core = 4
R2 = 2 * R                # merged LoRA rank = 128
P = 128
DC = D // P               # 10 D chunks
SB = 512                  # sub-block rows (PSUM fp32 bank = 512)
NST = SB // P             # 4 S-subtiles per sub-block
NSUB = NS * S // SB       # 32 sub-blocks per core
DBLK = 1024               # DMA granularity rows
SPB = DBLK // SB          # sub-blocks per DMA block = 2

F32 = mybir.dt.float32
BF16 = mybir.dt.bfloat16

_CACHED = {}


def _build_module():
    nc = bacc.Bacc(None, target_bir_lowering=False)

    # xt[b, k] = x[b].T[k*128:(k+1)*128, :]    ([128 d-part, S])
    xt = nc.dram_tensor("xt", [NS, DC, P, S], BF16, kind="ExternalInput")
    # acat[b, p, k, r] = Acat[b].T[k*128 + p, r] ([128 d-part, DC, R2])
    acat = nc.dram_tensor("acat", [NS, P, DC, R2], BF16, kind="ExternalInput")
    # bct[b] = (ALPHA * Bcat[b]).T             ([128 r-part, D])
    bct = nc.dram_tensor("bct", [NS, P, D], BF16, kind="ExternalInput")
    out = nc.dram_tensor("out", [NS, S, D], BF16, kind="ExternalOutput")

    with tile.TileContext(nc) as tc:
        with (
            tc.tile_pool(name="const", bufs=1) as constp,
            tc.tile_pool(name="xin", bufs=5) as xin_p,
            tc.tile_pool(name="ht", bufs=4) as ht_p,
            tc.tile_pool(name="osb", bufs=4) as out_p,
            tc.tile_pool(name="h_ps", bufs=2, space="PSUM") as h_ps,
            tc.tile_pool(name="o_ps", bufs=2, space="PSUM") as o_ps,
        ):
            acat_sb = constp.tile([P, NS, DC, R2], BF16)
            bct_sb = constp.tile([P, NS, D], BF16)
            # sample 0's tables first so compute can start ASAP (one per
            # ring); the rest queue behind the first x blocks.
            nc.sync.dma_start(acat_sb[:, 0], acat[0])
            nc.scalar.dma_start(bct_sb[:, 0], bct[0])

            prev = None
            for n in range(NSUB + 1):
                if n < NSUB:
                    b = n // (S // SB)
                    j = n % (S // SB)          # sub-block within sample
                    m = n // SPB               # DMA-block index
                    di = m % (S // DBLK)       # DMA-block within sample
                    si = n % SPB               # sub-block within DMA block
                    if si == 0:
                        x_nat = xin_p.tile([P, DC, DBLK], BF16, tag="xin")
                        src = xt[b, :, :, ts(di, DBLK)].rearrange("k p s -> p k s")
                        if m == 0:
                            # pipeline fill: no out stream yet -- split the
                            # first block across both HWDGE rings.
                            nc.sync.dma_start(x_nat[:, 0:5], src[:, 0:5])
                            nc.scalar.dma_start(x_nat[:, 5:10], src[:, 5:10])
                        elif m == 1:
                            nc.scalar.dma_start(x_nat[:], src)
                        else:
                            nc.sync.dma_start(x_nat[:], src)
                    # prefetch the next sample's tables one sample ahead
                    if j == 0 and b + 1 < NS:
                        nc.sync.dma_start(acat_sb[:, b + 1], acat[b + 1])
                        nc.sync.dma_start(bct_sb[:, b + 1], bct[b + 1])
                    if si == 0:
                        out_sb = out_p.tile([P, SPB * NST, D], BF16, tag="osb")

                    # GEMM1: hT[r, s] accumulated over D chunks
                    hp = h_ps.tile([P, SB], F32, tag="hp")
                    for k in range(DC):
                        nc.tensor.matmul(
                            hp[:],
                            acat_sb[:, b, k],
                            x_nat[:, k, ts(si, SB)],
                            start=(k == 0),
                            stop=(k == DC - 1),
                        )
                    ht = ht_p.tile([P, SB], BF16, tag="ht")
                    nc.scalar.copy(ht[:], hp[:])

                # GEMM2 for the previous sub-block (one-stage software
                # pipeline so the PE never waits on the ht evacuation).
                if prev is not None:
                    pht, pb, posb, pn = prev
                    for st in range(NST):
                        # 3-bank PSUM tile; each matmul stays in one bank.
                        op = o_ps.tile([P, 1536], F32, tag="op")
                        for n0, nsz in ((0, 512), (512, 512), (1024, 256)):
                            nc.tensor.matmul(
                                op[:, n0 : n0 + nsz],
                                pht[:, ts(st, P)],
                                bct_sb[:, pb, n0 : n0 + nsz],
                                start=True,
                                stop=True,
                            )
                        psi = pn % SPB
                        dst = posb[:, psi * NST + st]
                        if st % 2 == 0:
                            nc.vector.tensor_copy(dst, op[:, 0:D])
                        else:
                            nc.scalar.copy(dst, op[:, 0:D])
                    if pn % SPB == SPB - 1:
                        pdi = (pn // SPB) % (S // DBLK)
                        dstd = out[pb, ts(pdi, DBLK)].rearrange(
                            "(st p) d -> p st d", p=P
                        )
                        if pn == NSUB - 1:
                            # pipeline drain: the in stream is done -- split
                            # the last block across both HWDGE rings.
                            nc.scalar.dma_start(dstd[:, 0:4], posb[:, 0:4])
                            nc.sync.dma_start(dstd[:, 4:8], posb[:, 4:8])
                        else:
                            nc.scalar.dma_start(dstd, posb[:])

                prev = (ht, b, out_sb, n) if n < NSUB else None

    nc.finalize()
    return nc


def _get_module():
    if "v3" not in _CACHED:
        _CACHED["v3"] = _build_module()
    return _CACHED["v3"]


def _prepare_in_maps(x, weight, A_experts, B_experts, A_gen, B_gen, label):
    x = np.asarray(x, dtype=np.float32)
    A_experts = np.asarray(A_experts, dtype=np.float32)
    B_experts = np.asarray(B_experts, dtype=np.float32)
    A_gen = np.asarray(A_gen, dtype=np.float32)
    B_gen = np.asarray(B_gen, dtype=np.float32)
    label = np.asarray(label).astype(np.int64)

    Ae = A_experts[label]                                    # [B, R, D]
    Be = B_experts[label]                                    # [B, D, R]
    Acat = np.concatenate(
        [Ae, np.broadcast_to(A_gen, (B, R, D))], axis=1
    )                                                        # [B, 2R, D]
    BctT = ALPHA * np.concatenate(
        [Be, np.broadcast_to(B_gen, (B, D, R))], axis=2
    ).transpose(0, 2, 1)                                     # [B, R2, D]

    acat = np.ascontiguousarray(
        Acat.transpose(0, 2, 1).reshape(B, DC, P, R2).transpose(0, 2, 1, 3)
    ).astype(ml_dtypes.bfloat16)                             # [B, P, DC, R2]
    bct = np.ascontiguousarray(BctT).astype(ml_dtypes.bfloat16)

    xb = x.astype(ml_dtypes.bfloat16)
    xt = np.ascontiguousarray(
        xb.reshape(B, S, DC, P).transpose(0, 2, 3, 1)
    )                                                        # [B, DC, P, S]

    in_maps = []
    for c in range(NCORES):
        sl = slice(c * NS, (c + 1) * NS)
        in_maps.append({"xt": xt[sl], "acat": acat[sl], "bct": bct[sl]})
    return in_maps


def _run(trace=False, **inputs):
    nc = _get_module()
    in_maps = _prepare_in_maps(**inputs)
    res = run_bass_kernel_spmd(
        nc, in_maps, core_ids=list(range(NCORES)), trace=trace
    )
    out = np.concatenate(
        [np.asarray(res.results[c]["out"]) for c in range(NCORES)], axis=0
    ).astype(np.float32)
    # torch loop runs range(B-1): the last sample's output stays zero
    out[B - 1] = 0.0
    return out, res


def kernel(**inputs) -> np.ndarray:
    out, _ = _run(trace=False, **inputs)
    return out


def kernel_traced(mode=None, **inputs):
    """Returns (out, BassKernelResults) with HW profile info."""
    return _run(trace=True, **inputs)


# revision 9
# speedup vs baseline: 1.0159x; 1.0159x over previous
"""MoE-LoRA kernel for Trainium2 (8 NeuronCores, Bass/Tile). v3

Math: per sample b (except the last, which is zeroed), with e = label[b]:
    out[b] = ALPHA * ( (x[b] @ A_e.T) @ B_e.T  +  (x[b] @ A_gen.T) @ B_gen.T )
The expert and general LoRA paths merge into a single rank-128 LoRA:
    Acat[b] = [A_e ; A_gen]          [2R, D]
    Bcat[b] = [B_e , B_gen]          [D, 2R]
    out[b]  = (x[b] @ Acat[b].T) @ (ALPHA * Bcat[b]).T

Key idea (v2): the host ships x PRE-TRANSPOSED (d-major bf16), so the device
runs only the two GEMM passes -- no PE transposes.  The kernel is DMA-bound:
~86 MB/core at ~400 GB/s.

v3: 512-row DMA granularity and edge ring-splitting -- during pipeline fill
only the input stream exists and during drain only the output stream, so
those phases use BOTH HWDGE rings (sync + scalar) to keep the SDMA pool fed.

Device pipeline per 512-row sub-block (software-pipelined by one stage):
    DMA xT sub-block in ([128 d-part, 10 k, 512 s])
    PE  GEMM1: hT[r=128, 512] += acatT_k.T @ xT_k      (10 D-chunks, PSUM)
    ACT evacuate hT -> SBUF bf16
    PE  GEMM2: out[s,d] = hT_st.T @ bctT   (N=512/512/256 into 3-bank PSUM)
    DVE/ACT evacuate out tiles -> SBUF bf16 (alternating)
    DMA out (512-row granularity)

Sharding: data-parallel over batch, 4 samples/core; per-sample LoRA tables
gathered + merged + transposed host-side, shipped bf16.
"""

import numpy as np
import ml_dtypes

import concourse.mybir as mybir
import concourse.tile as tile
from concourse import bacc
from concourse.bass import ts
from concourse.bass_utils import run_bass_kernel_spmd

# Problem shape (hardcoded; kernel.py must be self-contained).
B, S, D, R, E = 32, 4096, 1280, 64, 8
ALPHA = 2.0
NCORES = 8
NS = B // NCORES          # samples per core = 4
R2 = 2 * R                # merged LoRA rank = 128
P = 128
DC = D // P               # 10 D chunks
SB = 512                  # sub-block rows (PSUM fp32 bank = 512)
NST = SB // P             # 4 S-subtiles per sub-block
NSUB = NS * S // SB       # 32 sub-blocks per core
DBLK = 1024               # DMA granularity rows
SPB = DBLK // SB          # sub-blocks per DMA block = 2

F32 = mybir.dt.float32
BF16 = mybir.dt.bfloat16

_CACHED = {}


def _build_module():
    nc = bacc.Bacc(None, target_bir_lowering=False)

    # xt[b, k] = x[b].T[k*128:(k+1)*128, :]    ([128 d-part, S])
    xt = nc.dram_tensor("xt", [NS, DC, P, S], BF16, kind="ExternalInput")
    # acat[b, p, k, r] = Acat[b].T[k*128 + p, r] ([128 d-part, DC, R2])
    acat = nc.dram_tensor("acat", [NS, P, DC, R2], BF16, kind="ExternalInput")
    # bct[b] = (ALPHA * Bcat[b]).T             ([128 r-part, D])
    bct = nc.dram_tensor("bct", [NS, P, D], BF16, kind="ExternalInput")
    out = nc.dram_tensor("out", [NS, S, D], BF16, kind="ExternalOutput")

    with tile.TileContext(nc) as tc:
        with (
            tc.tile_pool(name="const", bufs=1) as constp,
            tc.tile_pool(name="xin", bufs=5) as xin_p,
            tc.tile_pool(name="ht", bufs=4) as ht_p,
            tc.tile_pool(name="osb", bufs=4) as out_p,
            tc.tile_pool(name="h_ps", bufs=2, space="PSUM") as h_ps,
            tc.tile_pool(name="o_ps", bufs=2, space="PSUM") as o_ps,
        ):
            acat_sb = constp.tile([P, NS, DC, R2], BF16)
            bct_sb = constp.tile([P, NS, D], BF16)
            # sample 0's tables first so compute can start ASAP (one per
            # ring); the rest queue behind the first x blocks.
            nc.sync.dma_start(acat_sb[:, 0], acat[0])
            nc.scalar.dma_start(bct_sb[:, 0], bct[0])

            prev = None
            for n in range(NSUB + 1):
                if n < NSUB:
                    b = n // (S // SB)
                    j = n % (S // SB)          # sub-block within sample
                    m = n // SPB               # DMA-block index
                    di = m % (S // DBLK)       # DMA-block within sample
                    si = n % SPB               # sub-block within DMA block
                    if si == 0:
                        x_nat = xin_p.tile([P, DC, DBLK], BF16, tag="xin")
                        src = xt[b, :, :, ts(di, DBLK)].rearrange("k p s -> p k s")
                        if m == 0:
                            # pipeline fill: no out stream yet -- split the
                            # first block across both HWDGE rings.
                            nc.sync.dma_start(x_nat[:, 0:5], src[:, 0:5])
                            nc.scalar.dma_start(x_nat[:, 5:10], src[:, 5:10])
                        else:
                            nc.sync.dma_start(x_nat[:], src)
                    # prefetch the next sample's tables one sample ahead
                    if j == 0 and b + 1 < NS:
                        nc.sync.dma_start(acat_sb[:, b + 1], acat[b + 1])
                        nc.sync.dma_start(bct_sb[:, b + 1], bct[b + 1])
                    if si == 0:
                        out_sb = out_p.tile([P, SPB * NST, D], BF16, tag="osb")

                    # GEMM1: hT[r, s] accumulated over D chunks
                    hp = h_ps.tile([P, SB], F32, tag="hp")
                    for k in range(DC):
                        nc.tensor.matmul(
                            hp[:],
                            acat_sb[:, b, k],
                            x_nat[:, k, ts(si, SB)],
                            start=(k == 0),
                            stop=(k == DC - 1),
                        )
                    ht = ht_p.tile([P, SB], BF16, tag="ht")
                    nc.scalar.copy(ht[:], hp[:])

                # GEMM2 for the previous sub-block (one-stage software
                # pipeline so the PE never waits on the ht evacuation).
                if prev is not None:
                    pht, pb, posb, pn = prev
                    for st in range(NST):
                        # 3-bank PSUM tile; each matmul stays in one bank.
                        op = o_ps.tile([P, 1536], F32, tag="op")
                        for n0, nsz in ((0, 512), (512, 512), (1024, 256)):
                            nc.tensor.matmul(
                                op[:, n0 : n0 + nsz],
                                pht[:, ts(st, P)],
                                bct_sb[:, pb, n0 : n0 + nsz],
                                start=True,
                                stop=True,
                            )
                        psi = pn % SPB
                        dst = posb[:, psi * NST + st]
                        if st % 2 == 0:
                            nc.vector.tensor_copy(dst, op[:, 0:D])
                        else:
                            nc.scalar.copy(dst, op[:, 0:D])
                    if pn % SPB == SPB - 1:
                        pdi = (pn // SPB) % (S // DBLK)
                        dstd = out[pb, ts(pdi, DBLK)].rearrange(
                            "(st p) d -> p st d", p=P
                        )
                        if pn == NSUB - 1:
                            # pipeline drain: the in stream is done -- split
                            # the last block across both HWDGE rings.
                            nc.scalar.dma_start(dstd[:, 0:4], posb[:, 0:4])
                            nc.sync.dma_start(dstd[:, 4:8], posb[:, 4:8])
                        else:
                            nc.scalar.dma_start(dstd, posb[:])

                prev = (ht, b, out_sb, n) if n < NSUB else None

    nc.finalize()
    return nc


def _get_module():
    if "v3" not in _CACHED:
        _CACHED["v3"] = _build_module()
    return _CACHED["v3"]


def _prepare_in_maps(x, weight, A_experts, B_experts, A_gen, B_gen, label):
    x = np.asarray(x, dtype=np.float32)
    A_experts = np.asarray(A_experts, dtype=np.float32)
    B_experts = np.asarray(B_experts, dtype=np.float32)
    A_gen = np.asarray(A_gen, dtype=np.float32)
    B_gen = np.asarray(B_gen, dtype=np.float32)
    label = np.asarray(label).astype(np.int64)

    Ae = A_experts[label]                                    # [B, R, D]
    Be = B_experts[label]                                    # [B, D, R]
    Acat = np.concatenate(
        [Ae, np.broadcast_to(A_gen, (B, R, D))], axis=1
    )                                                        # [B, 2R, D]
    BctT = ALPHA * np.concatenate(
        [Be, np.broadcast_to(B_gen, (B, D, R))], axis=2
    ).transpose(0, 2, 1)                                     # [B, R2, D]

    acat = np.ascontiguousarray(
        Acat.transpose(0, 2, 1).reshape(B, DC, P, R2).transpose(0, 2, 1, 3)
    ).astype(ml_dtypes.bfloat16)                             # [B, P, DC, R2]
    bct = np.ascontiguousarray(BctT).astype(ml_dtypes.bfloat16)

    xb = x.astype(ml_dtypes.bfloat16)
    xt = np.ascontiguousarray(
        xb.reshape(B, S, DC, P).transpose(0, 2, 3, 1)
    )                                                        # [B, DC, P, S]

    in_maps = []
    for c in range(NCORES):
        sl = slice(c * NS, (c + 1) * NS)
        in_maps.append({"xt": xt[sl], "acat": acat[sl], "bct": bct[sl]})
    return in_maps


def _run(trace=False, **inputs):
    nc = _get_module()
    in_maps = _prepare_in_maps(**inputs)
    res = run_bass_kernel_spmd(
        nc, in_maps, core_ids=list(range(NCORES)), trace=trace
    )
    out = np.concatenate(
        [np.asarray(res.results[c]["out"]) for c in range(NCORES)], axis=0
    ).astype(np.float32)
    # torch loop runs range(B-1): the last sample's output stays zero
    out[B - 1] = 0.0
    return out, res


def kernel(**inputs) -> np.ndarray:
    out, _ = _run(trace=False, **inputs)
    return out


def kernel_traced(mode=None, **inputs):
    """Returns (out, BassKernelResults) with HW profile info."""
    return _run(trace=True, **inputs)
